# revision 37
# baseline (speedup 1.0000x reference)
"""Trainium2 Bass kernel for the entity-assignment loss.

Math: per sample b, C[i,j] = mean_d (yt[b,i,d]-yp[b,j,d])^2.
loss = mean_b ( min_perm sum_i C[i, perm(i)] / 8 ).

Since each permutation uses every row i and every column j exactly once,
  sum_i C[i, perm(i)] = (nt + np - 2 * sum_i dot(i, perm(i))) / 64
with nt = sum_i |yt_i|^2, np = sum_j |yp_j|^2 (per-sample constants).
So min over perms only needs MAX over perms of the dot sum, computed with a
2^8 bitmask DP whose bit-i update is a perfectly strided access pattern.

Perf notes (measured on TRN2):
- every DVE instruction pays a ~70-130ns issue/SBUF-access overhead, and
  scalar_tensor_tensor runs at 1x (no DVE fast modes). 96 DP ops is provably
  minimal (6 inner steps x 8 bits x 2 sample chunks; chunks cannot merge
  because the stt scalar is per-partition and two samples share each
  partition), but each op's range is trimmed to the states whose popcount
  can matter at that step, cutting DP exec ~31%.
- GpSimd cannot help: TensorScalarPtr/TensorTensor are illegal opcodes on
  Pool in the TRN2 NEFF codegen (only Memset/DMA/custom-ISA kernels run
  there), so Pool only does the NEG memsets, overlapped with the loads.
- inputs are pre-cast to fp16 on the host: halves DMA bytes and removes the
  ScalarE cast + act-table load from the critical path; SQUARE norms on
  ScalarE overlap the DVE multiply.
- yt and yp are host-packed into ONE interleaved DRAM buffer loaded as two
  per-chunk DMAs, so the first multiply waits on a single DMA completion and
  starts ~0.3us earlier than with per-tensor loads.
- the fold tree (2x mode) replaces the segmented tensor_reduce (1x).

Sharding: pure data parallelism, 256 samples per core across 8 cores; the
final mean is taken on the host from per-sample partial results.
"""

import os
import sys

if "/opt/trn_rl_repo" not in sys.path:
    sys.path.insert(0, "/opt/trn_rl_repo")

import numpy as np

B, N, D = 2048, 8, 64
N_CORES = 8
B_LOC = B // N_CORES        # 256 samples per core
NT = 2                      # two samples per partition row (free-dim chunks)
NEG = -60000.0              # fp16-safe "minus infinity"

TRACE = False
_CACHE = {}


def _build():
    import concourse.bacc as bacc
    import concourse.mybir as mybir
    from concourse.tile import TileContext

    f32 = mybir.dt.float32
    f16 = mybir.dt.float16
    Alu = mybir.AluOpType
    Act = mybir.ActivationFunctionType

    nc = bacc.Bacc("TRN2", target_bir_lowering=False, debug=False)
    # one packed input: row p = [yp(2p) | yt(2p) | yp(2p+1) | yt(2p+1)],
    # fp16 — so each chunk's full data arrives via a single DMA
    ytp_d = nc.declare_dram_parameter("ytp", [128, 2 * NT * N * D], f16,
                                      isOutput=False)
    out_d = nc.declare_dram_parameter("out", [128, NT], f32, isOutput=True)

    with TileContext(nc) as tc:
        with (
            tc.tile_pool(name="io", bufs=1) as io_pool,
            tc.tile_pool(name="work", bufs=2) as work_pool,
            tc.tile_pool(name="res", bufs=1) as res_pool,
        ):
            loss_t = res_pool.tile([128, NT], f32, tag="loss")
            s_all = res_pool.tile([128, NT], f32, tag="s_all")
            G32 = res_pool.tile([128, NT * N * N], f32, tag="G32")
            dpa = res_pool.tile([128, NT * 256], f16, tag="dpa")
            dpb = res_pool.tile([128, NT * 256], f16, tag="dpb")
            cand = res_pool.tile([128, NT * N], f16, tag="cand")

            # quarter-granularity loads: chunk-0 halves first so the first
            # multiply can start before chunk-1 data lands
            ytp_t = io_pool.tile([128, 2 * NT * N * D], f16, tag="ytp")
            # two half-loads on SP's HWDGE (measured fastest issue path):
            # chunk-0 data lands ~0.6us before chunk-1's
            HW = N * D
            nc.sync.dma_start(out=ytp_t[:, 0:2 * HW], in_=ytp_d[:, 0:2 * HW])
            nc.sync.dma_start(out=ytp_t[:, 2 * HW:4 * HW],
                              in_=ytp_d[:, 2 * HW:4 * HW])

            # DP state init on GpSimd, overlapped with the input DMA
            nc.gpsimd.memset(dpa[:, :], NEG)
            nc.gpsimd.memset(dpb[:, :], NEG)

            # G matrices, both chunks in each op (halves the per-instruction
            # SBUF-access bubbles): broadcast multiply, three binary folds
            # over d, then a segmented reduce; norms on ScalarE in parallel
            # packed layout: yp_h|yt_h are contiguous, so ONE Square+accum
            # per chunk yields nt+np = s_all directly; the result is DMAed
            # out early, hidden under the DP
            for h in range(NT):
                sq = work_pool.tile([128, 2 * N * D], f32, tag="sq")
                nc.scalar.activation(out=sq[:, :],
                                     in_=ytp_t[:, 2 * h * HW:(2 * h + 2) * HW],
                                     func=Act.Square,
                                     accum_out=s_all[:, h:h + 1])

            # one multiply per chunk: chunk 0 starts as soon as its two
            # quarter-loads land, before chunk-1 data arrives
            prod = work_pool.tile([128, NT * N * N * D], f16, tag="prod")
            for h in range(NT):
                yt_b = ytp_t[:, (2 * h + 1) * HW:(2 * h + 2) * HW] \
                    .rearrange("p (i d) -> p i d", d=D).unsqueeze(2) \
                    .broadcast_to([128, N, N, D])
                yp_b = ytp_t[:, 2 * h * HW:(2 * h + 1) * HW] \
                    .rearrange("p (j d) -> p j d", d=D).unsqueeze(1) \
                    .broadcast_to([128, N, N, D])
                nc.vector.tensor_tensor(
                    out=prod[:, h * N * N * D:(h + 1) * N * N * D]
                        .rearrange("p (i j d) -> p i j d", j=N, d=D),
                    in0=yt_b, in1=yp_b, op=Alu.mult)
            pv = prod.rearrange("p (q d) -> p q d", d=D)
            half = work_pool.tile([128, NT * N * N * D // 2], f16, tag="half")
            hv = half.rearrange("p (q d) -> p q d", d=D // 2)
            nc.vector.tensor_tensor(
                out=hv, in0=pv[:, :, 0:D // 2], in1=pv[:, :, D // 2:D],
                op=Alu.add)
            quart = work_pool.tile([128, NT * N * N * D // 4], f16, tag="quart")
            qv = quart.rearrange("p (q d) -> p q d", d=D // 4)
            nc.vector.tensor_tensor(
                out=qv, in0=hv[:, :, 0:D // 4], in1=hv[:, :, D // 4:D // 2],
                op=Alu.add)
            eighth = work_pool.tile([128, NT * N * N * D // 8], f16, tag="eighth")
            ev = eighth.rearrange("p (q d) -> p q d", d=D // 8)
            nc.vector.tensor_tensor(
                out=ev, in0=qv[:, :, 0:D // 8], in1=qv[:, :, D // 8:D // 4],
                op=Alu.add)
            # finish with three more folds instead of a segmented
            # tensor_reduce: the folds run in DVE 2x mode, the reduce is 1x
            s16 = work_pool.tile([128, NT * N * N * D // 16], f16, tag="s16")
            sv = s16.rearrange("p (q d) -> p q d", d=D // 16)
            nc.vector.tensor_tensor(
                out=sv, in0=ev[:, :, 0:D // 16], in1=ev[:, :, D // 16:D // 8],
                op=Alu.add)
            s32 = work_pool.tile([128, NT * N * N * D // 32], f16, tag="s32")
            wv = s32.rearrange("p (q d) -> p q d", d=D // 32)
            nc.vector.tensor_tensor(
                out=wv, in0=sv[:, :, 0:D // 32], in1=sv[:, :, D // 32:D // 16],
                op=Alu.add)
            nc.vector.tensor_tensor(
                out=G32.rearrange("p (q e) -> p q e", e=1),
                in0=wv[:, :, 0:1], in1=wv[:, :, 1:2], op=Alu.add)

            # bitmask DP over both chunks: states laid out [chunk, state]
            g_v = G32.rearrange("p (h q) -> p h q", h=NT)
            bufs = [dpa, dpb]
            for k in range(N):
                old = bufs[k % 2]
                new = bufs[(k + 1) % 2]
                if k == 0:
                    # singletons, pairwise-merged: targets {2^i, 2^(i+1)}
                    # are stride-2^i; G cols {i*8, (i+1)*8} are stride-8.
                    # Summing the two f5-level halves here (instead of
                    # copying from the f6 output) moves the dependency two
                    # ops back, hiding the cross-op semaphore stall.
                    wv_v = s32.rearrange("p (h q d) -> p h q d", h=NT, d=2)
                    for i in range(0, N, 2):
                        ci = 2 ** i
                        nv = new.rearrange("p (h s) -> p h s", h=NT)
                        tgt = nv[:, :, ci:2 * ci + 1:ci]
                        sl = slice(i * N, (i + 2) * N, N)
                        nc.vector.tensor_tensor(
                            out=tgt, in0=wv_v[:, :, sl, 0],
                            in1=wv_v[:, :, sl, 1], op=Alu.add)
                    continue
                if k == N - 1:
                    # final column: collect the 8 candidates densely; cand
                    # slot order ascends with source state (reduce_max is
                    # order-invariant).
                    for i in range(0, N, 2):
                        ci = 2 ** i
                        ov = old.rearrange("p (h s) -> p h s", h=NT)
                        src = ov[:, :, 255 - 2 * ci:256 - ci:ci]
                        cv = cand.rearrange("p (h s) -> p h s", h=NT)[:, :, i:i + 2]
                        gsrc = g_v[:, :, (i + 1) * N + k::-N][:, :, 0:2]
                        nc.vector.tensor_tensor(out=cv, in0=src, in1=gsrc,
                                                op=Alu.add)
                    continue
                # inner steps, all on DVE (no other engine can run
                # TensorScalarPtr/TensorTensor on TRN2)
                for i in range(N):
                    ci = 2 ** i
                    col = i * N + k
                    a = 256 // (2 * ci)
                    # popcount range trim: at step k only targets with
                    # popcount k+1 matter. With state = hi*(2ci) + bit_i*ci
                    # + lo, restrict hi/lo to the value range that covers
                    # every split of the k other bits across the 7-i high
                    # and i low positions. Unwritten slots keep NEG/stale
                    # values, which stay valid lower bounds.
                    hi_bits, lo_bits = N - 1 - i, i

                    def _rng(m, rest):
                        # value range of an m-bit segment whose popcount can
                        # be [max(0, k-rest), min(k, m)]
                        return (2 ** max(0, k - rest) - 1,
                                2 ** m - 2 ** (m - min(k, m)))

                    if k in (1, N - 2) and (lo_bits == 0 or hi_bits == 0):
                        # near-boundary steps for the edge bits: splitting
                        # the single wide bit field in two tightens the
                        # cover; stt allows only 2 free dims, so this only
                        # fits when the other side has no bits
                        if hi_bits >= lo_bits:
                            sb = hi_bits // 2
                            st = hi_bits - sb
                            t0, t1 = _rng(st, sb + lo_bits)
                            b0, b1 = _rng(sb, st + lo_bits)
                            l0, l1 = _rng(lo_bits, hi_bits) if lo_bits \
                                else (0, 0)
                            vo = old.rearrange(
                                "p (h at ab b c) -> p h at ab b c",
                                h=NT, at=2 ** st, ab=2 ** sb, b=2)
                            src = vo[:, :, t0:t1 + 1, b0:b1 + 1, 0,
                                     l0:l1 + 1]
                            vn = new.rearrange(
                                "p (h at ab b c) -> p h at ab b c",
                                h=NT, at=2 ** st, ab=2 ** sb, b=2)
                            tgt = vn[:, :, t0:t1 + 1, b0:b1 + 1, 1,
                                     l0:l1 + 1]
                        else:
                            sb = lo_bits // 2
                            st = lo_bits - sb
                            a0, a1 = _rng(hi_bits, lo_bits) if hi_bits \
                                else (0, 0)
                            t0, t1 = _rng(st, sb + hi_bits)
                            b0, b1 = _rng(sb, st + hi_bits)
                            vo = old.rearrange(
                                "p (h a b ct cb) -> p h a b ct cb",
                                h=NT, b=2, ct=2 ** st, cb=2 ** sb)
                            src = vo[:, :, a0:a1 + 1, 0, t0:t1 + 1,
                                     b0:b1 + 1]
                            vn = new.rearrange(
                                "p (h a b ct cb) -> p h a b ct cb",
                                h=NT, b=2, ct=2 ** st, cb=2 ** sb)
                            tgt = vn[:, :, a0:a1 + 1, 1, t0:t1 + 1,
                                     b0:b1 + 1]
                    else:
                        hi0, hi1 = _rng(hi_bits, lo_bits)
                        lo0, lo1 = _rng(lo_bits, hi_bits) if i > 0 else (0, 0)
                        vo = old.rearrange("p (h a b c) -> p h a b c",
                                           h=NT, b=2, c=ci)
                        src = vo[:, :, hi0:hi1 + 1, 0, lo0:lo1 + 1]
                        vn = new.rearrange("p (h a b c) -> p h a b c",
                                           h=NT, b=2, c=ci)
                        tgt = vn[:, :, hi0:hi1 + 1, 1, lo0:lo1 + 1]
                    for h in range(NT):
                        nc.vector.scalar_tensor_tensor(
                            out=tgt[:, h], in0=src[:, h],
                            scalar=G32[:, h * N * N + col:h * N * N + col + 1],
                            in1=tgt[:, h], op0=Alu.add, op1=Alu.max)

            dmax = res_pool.tile([128, NT], f16, tag="dmax")
            nc.vector.tensor_reduce(
                out=dmax[:, :],
                in_=cand.rearrange("p (h s) -> p h s", h=NT),
                axis=mybir.AxisListType.X, op=Alu.max)
            nc.vector.scalar_tensor_tensor(
                out=loss_t[:, :],
                in0=dmax[:, :],
                scalar=-2.0,
                in1=s_all[:, :],
                op0=Alu.mult,
                op1=Alu.add,
            )
            nc.sync.dma_start(out=out_d[:, :], in_=loss_t[:, :])
    nc.compile()
    return nc


def kernel(y_true: np.ndarray, y_pred: np.ndarray) -> np.ndarray:
    from concourse.bass_utils import run_bass_kernel_spmd

    if "nc" not in _CACHE:
        _CACHE["nc"] = _build()
    nc = _CACHE["nc"]

    yt = np.asarray(y_true, dtype=np.float16).reshape(B, N * D)
    yp = np.asarray(y_pred, dtype=np.float16).reshape(B, N * D)

    def pack(c):
        # row p = [yp(2p) | yt(2p) | yp(2p+1) | yt(2p+1)]
        ytc = yt[c * B_LOC:(c + 1) * B_LOC].reshape(128, 2, N * D)
        ypc = yp[c * B_LOC:(c + 1) * B_LOC].reshape(128, 2, N * D)
        arr = np.empty((128, 4, N * D), dtype=np.float16)
        arr[:, 0] = ypc[:, 0]
        arr[:, 1] = ytc[:, 0]
        arr[:, 2] = ypc[:, 1]
        arr[:, 3] = ytc[:, 1]
        return np.ascontiguousarray(arr.reshape(128, 4 * N * D))

    in_maps = [{"ytp": pack(c)} for c in range(N_CORES)]
    res = run_bass_kernel_spmd(nc, in_maps, list(range(N_CORES)), trace=TRACE)
    _CACHE["last_results"] = res
    vals = np.concatenate([np.asarray(r["out"], dtype=np.float64).reshape(-1)
                           for r in res.results])
    loss = vals.mean() / (D * N)
    return np.float32(loss)
